# revision 23
# baseline (speedup 1.0000x reference)
"""Tacotron-style decoder step on 8 Trainium2 NeuronCores.

Strategy:
  Phase A (data-parallel over batch, 32/core): prenet, GRU, LSA attention,
    context vector. Everything kept in transposed [feature, batch] layout so
    per-partition ACT biases and f32r matmuls line up.
  AllGather #1: per-core [ctxT; h'T] (640x32) -> full xcat (5120x32).
  Phase B (model-parallel over LSTM units, 128/core): rnn_in, LSTM1,
    AllGather #2 (h1_new 128x256 -> 1024x256), LSTM2, AllGather #3, then
    mel/stop projections computed redundantly for the full batch.
Host side shards/transposes inputs and reassembles full outputs.
"""
import sys

sys.path.insert(0, "/opt/trn_rl_repo")

import numpy as np
from concourse import bacc, mybir, tile
from concourse import bass_utils

F32 = mybir.dt.float32
F32R = mybir.dt.float32r
AF = mybir.ActivationFunctionType
ALU = mybir.AluOpType
AX = mybir.AxisListType

B, T, DIN, DEC, LSTMD = 256, 400, 512, 128, 1024
NMELS, PRE, NFILT, KS = 80, 256, 32, 31
NCORES = 8
BL = B // NCORES        # 32 local batch
UL = LSTMD // NCORES    # 128 local LSTM units
TP = 416                # padded T for 32-blocks
ENC_BUFS = 6

_CACHE = {}


def _declare(nc):
    I = {}
    O = {}

    def di(name, shape, dt=F32R):
        I[name] = nc.dram_tensor(name, list(shape), dt, kind="ExternalInput").ap()

    def do(name, shape, dt=F32):
        O[name] = nc.dram_tensor(name, list(shape), dt, kind="ExternalOutput").ap()

    # replicated weights
    di("w1T", (NMELS, PRE))
    di("w2T", (PRE, PRE))
    di("gihT", (DIN + PRE, 3 * DEC))
    di("ghhT", (DEC, 3 * DEC))
    di("lsaWT", (DEC, DEC))
    di("LWT", (KS, DEC))
    di("ident", (DEC, DEC))
    di("VMSK", (DEC, BL * BL))
    di("rnnT", (DIN + DEC, LSTMD))
    di("melT", (LSTMD, NMELS))
    di("stopTx", (LSTMD, 1))
    di("stopTc", (DIN, 1))
    di("h1T", (LSTMD, B))
    di("h2T", (LSTMD, B))
    di("vecs", (DEC, 17), F32)
    # per-core
    di("pinT", (NMELS, BL))
    di("hT0", (DEC, BL))
    di("ctxT0", (DIN, BL))
    di("projT", (BL, DEC, T))
    di("enc", (BL, T, DIN))
    di("cumP", (BL, T + KS - 1))
    di("cum", (BL, T), F32)
    di("mask", (BL, T), F32)
    di("l1ihT", (LSTMD, 4 * UL))
    di("l1hhT", (LSTMD, 4 * UL))
    di("l2ihT", (LSTMD, 4 * UL))
    di("l2hhT", (LSTMD, 4 * UL))
    di("lb1", (UL, 4), F32)
    di("lb2", (UL, 4), F32)
    di("c1T", (UL, B), F32)
    di("c2T", (UL, B), F32)
    di("stopb", (1, 1), F32)
    # outputs
    do("scores", (BL, T))
    do("cumn", (BL, T))
    do("attnh", (BL, DEC), F32R)
    do("ctx", (BL, DIN), F32R)
    do("h1n", (UL, B), F32R)
    do("c1n", (UL, B))
    do("h2n", (UL, B), F32R)
    do("c2n", (UL, B))
    do("melsT", (NMELS, B))
    do("stopT", (1, B))
    return I, O


def _program(nc, tc, I, O, single=False):
    import dataclasses

    with (
        tc.tile_pool(name="const", bufs=1) as cp,
        tc.tile_pool(name="wp", bufs=1) as wp,
        tc.tile_pool(name="tps", bufs=2, space="PSUM") as tps,
        tc.tile_pool(name="atps", bufs=2, space="PSUM") as atps,
        tc.tile_pool(name="acc", bufs=1, space="PSUM") as accp,
        tc.tile_pool(name="gat", bufs=2, space="PSUM") as gatp,
        tc.tile_pool(name="melps", bufs=1, space="PSUM") as melpsp,
        tc.tile_pool(name="hw", bufs=2) as hwp,
        tc.tile_pool(name="lw", bufs=1) as lwp,
        tc.tile_pool(name="dram", bufs=1, space="DRAM") as dp,
    ):
        # weights and other persistent tiles load on the ACT HWDGE queue so the
        # SP queue only carries latency-critical streaming DMAs.
        def load(pool, name, shape, dt=F32R, src=None, tag=None, eng=None):
            t = pool.tile(list(shape), dt, tag=tag or name)
            (eng or nc.gpsimd).dma_start(t[:], src if src is not None else I[name])
            return t

        def load_chunked(pool, name, width, nchunks, dt=F32R, tag=None, eng=None):
            """dram [nchunks*128, width] -> one sbuf tile [128, nchunks*width],
            chunk c at cols [width*c : width*(c+1)], via a single 3D-AP DMA."""
            t = pool.tile([128, nchunks * width], dt, tag=tag or name, name=tag or name)
            srcd = I[name]
            src = dataclasses.replace(
                srcd[0:1, 0:1],
                offset=srcd.offset,
                ap=[[width, 128], [128 * width, nchunks], [1, width]],
            )
            (eng or nc.gpsimd).dma_start(t[:], src)
            return t

        w1T = load(cp, "w1T", (NMELS, PRE))
        w2T_all = load_chunked(cp, "w2T", PRE, 2)
        gih_all = load_chunked(cp, "gihT", 3 * DEC, 6)
        ghh = load(cp, "ghhT", (DEC, 3 * DEC))
        lsaWT = load(cp, "lsaWT", (DEC, DEC))
        LWT = load(cp, "LWT", (KS, DEC))
        ident = load(cp, "ident", (DEC, DEC))
        VMSK = load(cp, "VMSK", (DEC, BL * BL))
        vecs = load(cp, "vecs", (DEC, 17), F32)
        pinT = load(cp, "pinT", (NMELS, BL))
        hT0 = load(cp, "hT0", (DEC, BL))
        ctxT0_all = load_chunked(cp, "ctxT0", BL, 4)
        cum_sb = load(cp, "cum", (BL, T), F32)
        mask_sb = load(cp, "mask", (BL, T), F32)
        lb1 = load(cp, "lb1", (UL, 4), F32)
        lb2 = load(cp, "lb2", (UL, 4), F32)
        c1T = load(cp, "c1T", (UL, B), F32)
        c2T = load(cp, "c2T", (UL, B), F32)
        stopb = load(cp, "stopb", (1, 1), F32)
        rnnT_all = load_chunked(wp, "rnnT", LSTMD, 5)
        melT_all = load_chunked(wp, "melT", NMELS, 8)
        stopTx_all = load_chunked(wp, "stopTx", 1, 8)
        stopTc_all = load_chunked(wp, "stopTc", 1, 4)
        rnnT = [rnnT_all[:, LSTMD * c : LSTMD * (c + 1)] for c in range(5)]
        melT = [melT_all[:, NMELS * c : NMELS * (c + 1)] for c in range(8)]
        stopTx = [stopTx_all[:, c : c + 1] for c in range(8)]
        stopTc = [stopTc_all[:, c : c + 1] for c in range(4)]
        # persistent phase-A results (consumed in phase B)
        hTn = cp.tile([128, BL], F32R, tag="hTn", name="hTn")
        ctxT_t = cp.tile([128, 4 * BL], F32R, tag="ctxT", name="ctxT")
        ctxT = [ctxT_t[:, BL * c : BL * (c + 1)] for c in range(4)]

        def bias(col):
            return vecs[:, col : col + 1]

        # ================= PHASE A =================
        with (
            tc.tile_pool(name="wka", bufs=1) as wk,
            tc.tile_pool(name="wk3", bufs=3) as wk3,
            tc.tile_pool(name="patp", bufs=2) as patp,
            tc.tile_pool(name="encp", bufs=ENC_BUFS) as encp,
        ):
            # ---------- prenet ----------
            pout = []
            h1pre = []
            for c in range(2):
                ps = tps.tile([128, BL], F32, tag="tps")
                nc.tensor.matmul(ps[:], w1T[:, 128 * c : 128 * (c + 1)], pinT[:], start=True, stop=True)
                h = wk.tile([128, BL], F32R, tag=f"pre{c}")
                nc.scalar.activation(h[:], ps[:], AF.Relu, bias=bias(c))
                h1pre.append(h)
            for c in range(2):
                ps = tps.tile([128, BL], F32, tag="tps")
                for a in range(2):
                    nc.tensor.matmul(
                        ps[:],
                        w2T_all[:, PRE * a + 128 * c : PRE * a + 128 * (c + 1)],
                        h1pre[a][:],
                        start=(a == 0),
                        stop=(a == 1),
                    )
                h = wk.tile([128, BL], F32R, tag=f"pout{c}")
                nc.scalar.activation(h[:], ps[:], AF.Relu, bias=bias(2 + c))
                pout.append(h)

            xcat = [ctxT0_all[:, BL * c : BL * (c + 1)] for c in range(4)] + [p[:] for p in pout]

            # ---------- GRU ----------
            def gru_gate_psum(g):
                ps = tps.tile([128, BL], F32, tag="tps")
                for c in range(6):
                    nc.tensor.matmul(
                        ps[:], gih_all[:, 384 * c + 128 * g : 384 * c + 128 * (g + 1)], xcat[c],
                        start=(c == 0), stop=False,
                    )
                nc.tensor.matmul(
                    ps[:], ghh[:, 128 * g : 128 * (g + 1)], hT0[:], start=False, stop=True
                )
                return ps

            ps_r = gru_gate_psum(0)
            r_sb = wk.tile([128, BL], F32, tag="r_sb")
            nc.scalar.activation(r_sb[:], ps_r[:], AF.Tanh, bias=bias(4), scale=0.5)
            nc.vector.tensor_scalar(r_sb[:], r_sb[:], 0.5, 0.5, ALU.mult, ALU.add)

            ps_z = gru_gate_psum(1)
            z_sb = wk.tile([128, BL], F32, tag="z_sb")
            nc.scalar.activation(z_sb[:], ps_z[:], AF.Tanh, bias=bias(5), scale=0.5)
            nc.vector.tensor_scalar(z_sb[:], z_sb[:], 0.5, 0.5, ALU.mult, ALU.add)

            ps_in = tps.tile([128, BL], F32, tag="tps")
            for c in range(6):
                nc.tensor.matmul(
                    ps_in[:], gih_all[:, 384 * c + 256 : 384 * c + 384], xcat[c], start=(c == 0), stop=(c == 5)
                )
            ps_hn = tps.tile([128, BL], F32, tag="tps")
            nc.tensor.matmul(ps_hn[:], ghh[:, 256:384], hT0[:], start=True, stop=True)
            t1 = wk.tile([128, BL], F32, tag="t1")
            nc.scalar.activation(t1[:], ps_hn[:], AF.Identity, bias=bias(6))
            t2 = wk.tile([128, BL], F32, tag="t2")
            nc.vector.tensor_tensor(t2[:], r_sb[:], t1[:], op=ALU.mult)
            t3 = wk.tile([128, BL], F32, tag="t3")
            nc.vector.tensor_tensor(t3[:], ps_in[:], t2[:], op=ALU.add)
            n_sb = wk.tile([128, BL], F32, tag="n_sb")
            nc.scalar.activation(n_sb[:], t3[:], AF.Tanh, bias=bias(7))
            t4 = wk.tile([128, BL], F32, tag="t4")
            nc.vector.tensor_tensor(t4[:], hT0[:].bitcast(F32), n_sb[:], op=ALU.subtract)
            t5 = wk.tile([128, BL], F32, tag="t5")
            nc.vector.tensor_tensor(t5[:], z_sb[:], t4[:], op=ALU.mult)
            nc.vector.tensor_tensor(hTn[:], n_sb[:], t5[:], op=ALU.add)

            # attn_hidden output (transpose to [BL, DEC])
            attnh = wk.tile([BL, DEC], F32R, tag="attnh")
            for j in range(4):
                nc.vector.transpose(
                    attnh[0:32, 32 * j : 32 * (j + 1)].bitcast(F32),
                    hTn[32 * j : 32 * (j + 1), 0:32].bitcast(F32),
                )
            nc.sync.dma_start(O["attnh"], attnh[:])

            # ---------- processed query ----------
            ps_q = tps.tile([128, BL], F32, tag="tps")
            nc.tensor.matmul(ps_q[:], lsaWT[:], hTn[:], start=True, stop=True)
            qb = wk.tile([128, BL], F32, tag="qb")
            nc.scalar.activation(qb[:], ps_q[:], AF.Identity, bias=bias(8))

            # masked-score tiles: allocate + zero early (no deps)
            KCH = [128, 128, 128, 16]
            smsk = []
            for c in range(4):
                kk = 128 if c < 3 else 16
                sm = wk.tile([kk, BL * (BL + 1)], F32R, tag=f"smsk{c}", name=f"smsk{c}")
                nc.vector.memset(sm[:].bitcast(F32), 0.0)
                smsk.append(sm)

            # ---------- attention per-b ----------
            u_acc = accp.tile([BL, T], F32, tag="acc")
            for b in range(BL):
                gp, jp = divmod(b, 4)
                if jp == 0:
                    # patches for 4 b in one DMA: patg[k, j*T+t] = cumP[4gp+j, k+t]
                    patg = patp.tile([KS, 4 * T], F32R, tag="patg")
                    pw0 = dataclasses.replace(
                        I["cumP"][0:1, :],
                        offset=I["cumP"].offset + 4 * gp * (T + KS - 1),
                        ap=[[1, KS], [T + KS - 1, 4], [1, T]],
                    )
                    nc.scalar.dma_start(patg[:], pw0)
                g, j = divmod(b, 4)
                if j == 0:
                    projg = wk3.tile([DEC, 4 * T], F32R, tag="projg")
                    pw = dataclasses.replace(
                        I["projT"][0:1, 0:1, 0:1],
                        offset=I["projT"].offset + 4 * g * DEC * T,
                        ap=[[T, DEC], [DEC * T, 4], [1, T]],
                    )
                    nc.scalar.dma_start(projg[:], pw)
                ps_b = atps.tile([DEC, T], F32, tag="attps")
                nc.tensor.matmul(ps_b[:], LWT[:], patg[:, T * jp : T * (jp + 1)], start=True, stop=False)
                nc.tensor.matmul(ps_b[:], ident[:], projg[:, T * j : T * (j + 1)], start=False, stop=True)
                tnh = wk3.tile([DEC, T], F32R, tag="tnh")
                nc.scalar.activation(tnh[:], ps_b[:], AF.Tanh, bias=qb[:, b : b + 1])
                nc.tensor.matmul(
                    u_acc[:], VMSK[:, BL * b : BL * (b + 1)], tnh[:],
                    start=(b == 0), stop=(b == BL - 1),
                )

            # ---------- softmax over T (no max-sub: |u| <= sum|v| ~ 2) ----------
            um = wk.tile([BL, T], F32, tag="um")
            nc.vector.tensor_tensor(um[:], u_acc[:], mask_sb[:], op=ALU.mult)
            ex = wk.tile([BL, TP], F32, tag="ex")
            nc.vector.memset(ex[:, T:TP], 0.0)
            nc.scalar.activation(ex[:, 0:T], um[:], AF.Exp)
            ssum = wk.tile([BL, 1], F32, tag="ssum")
            nc.vector.tensor_reduce(ssum[:], ex[:, 0:T], axis=AX.X, op=ALU.add)
            rinv = wk.tile([BL, 1], F32, tag="rinv")
            nc.vector.reciprocal(rinv[:], ssum[:])
            sc = wk.tile([BL, T], F32, tag="sc")
            nc.vector.tensor_scalar(sc[:], ex[:, 0:T], rinv[:, 0:1], None, ALU.mult)
            nc.sync.dma_start(O["scores"], sc[:])
            cumn = wk.tile([BL, T], F32, tag="cumn")
            nc.vector.tensor_tensor(cumn[:], cum_sb[:], sc[:], op=ALU.add)
            nc.sync.dma_start(O["cumn"], cumn[:])

            # ---------- transposed (unnormalized) scores -> masked tiles ----------
            sct = []
            for c in range(3):
                t = wk.tile([128, 32], F32, tag=f"sct{c}")
                for j in range(4):
                    nc.vector.transpose(
                        t[32 * j : 32 * (j + 1), 0:32],
                        ex[0:32, 128 * c + 32 * j : 128 * c + 32 * (j + 1)],
                    )
                sct.append(t)
            t = wk.tile([32, 32], F32, tag="sct3")
            nc.vector.transpose(t[0:32, 0:32], ex[0:32, 384:416])
            sct.append(t)
            for c in range(4):
                kk = KCH[c]
                nc.vector.tensor_copy(
                    smsk[c][:, 0 : BL * (BL + 1) : BL + 1], sct[c][0:kk, 0:BL]
                )

            # ---------- context ----------
            ctx_acc = accp.tile([BL, DIN], F32, tag="acc")
            first = True
            for b in range(BL):
                # one [128, 3*DIN] DMA for chunks 0-2 + one [16, DIN] for chunk 3
                et = encp.tile([128, 4 * DIN], F32R, tag="enc")
                ew = dataclasses.replace(
                    I["enc"][0:1, 0:1, 0:1],
                    offset=I["enc"].offset + b * T * DIN,
                    ap=[[DIN, 128], [128 * DIN, 3], [1, DIN]],
                )
                nc.sync.dma_start(et[:, 0 : 3 * DIN], ew)
                nc.sync.dma_start(et[0:16, 3 * DIN : 4 * DIN], I["enc"][b, 384:400, :])
                for c in range(4):
                    kk = KCH[c]
                    nc.tensor.matmul(
                        ctx_acc[:], smsk[c][0:kk, BL * b : BL * (b + 1)],
                        et[0:kk, DIN * c : DIN * (c + 1)],
                        start=first, stop=(b == BL - 1 and c == 3),
                    )
                    first = False
            ctx_sb = wk.tile([BL, DIN], F32R, tag="ctx_sb")
            nc.vector.tensor_scalar(ctx_sb[:], ctx_acc[:], rinv[:, 0:1], None, ALU.mult)
            nc.sync.dma_start(O["ctx"], ctx_sb[:])
            for c in range(4):
                for j in range(4):
                    nc.vector.transpose(
                        ctxT_t[32 * j : 32 * (j + 1), BL * c : BL * c + 32].bitcast(F32),
                        ctx_sb[0:32, 128 * c + 32 * j : 128 * c + 32 * (j + 1)].bitcast(F32),
                    )

            # AllGather 1 inputs (inside phase A scope; reads ctxT/hTn from cp)
            cc1i = dp.tile([DIN + DEC, BL], F32R, tag="cc1i")
            dst = dataclasses.replace(
                cc1i[0:DIN, :],
                ap=[[BL, 128], [128 * BL, 4], [1, BL]],
            )
            nc.sync.dma_start(dst, ctxT_t[:])
            nc.sync.dma_start(cc1i[DIN : DIN + DEC, :], hTn[:])

        # ================= PHASE B =================
        cc1o = dp.tile([NCORES * (DIN + DEC), BL], F32R, tag="cc1o")
        if single:
            nc.sync.dma_start(cc1o[0 : DIN + DEC, :], cc1i[:, :])
        else:
            nc.gpsimd.collective_compute(
                "AllGather", ALU.bypass, replica_groups=[list(range(NCORES))],
                ins=[cc1i.opt()], outs=[cc1o.opt()],
            )
        with (
            tc.tile_pool(name="wkb", bufs=1) as wk,
            tc.tile_pool(name="xk", bufs=8) as xkp,
            tc.tile_pool(name="lw2", bufs=1) as lw2p,
        ):
            xcg_all = wk.tile([128, 5 * B], F32R, tag="xcg", name="xcg_all")
            full = cc1o[:, :]
            for cc in range(5):
                src = dataclasses.replace(
                    full,
                    offset=full.offset + 128 * cc * BL,
                    ap=[[BL, 128], [(DIN + DEC) * BL, NCORES], [1, BL]],
                )
                nc.sync.dma_start(xcg_all[:, B * cc : B * (cc + 1)], src)
            xcg = [xcg_all[:, B * cc : B * (cc + 1)] for cc in range(5)]

            # ---------- rnn_in: xT chunks ----------
            xT = []
            for m in range(8):
                ps = tps.tile([128, B], F32, tag="tps")
                for c in range(5):
                    nc.tensor.matmul(
                        ps[:], rnnT_all[:, LSTMD * c + 128 * m : LSTMD * c + 128 * (m + 1)], xcg[c],
                        start=(c == 0), stop=(c == 4),
                    )
                x = xkp.tile([128, B], F32R, tag="xt")
                nc.scalar.activation(x[:], ps[:], AF.Identity, bias=bias(9 + m))
                xT.append(x)

            # mel/stop partial accumulation (x part now; h parts later)
            mel_ps = melpsp.tile([NMELS, B], F32, tag="melp")
            for c in range(8):
                nc.tensor.matmul(mel_ps[:], melT[c], xT[c][:],
                                 start=(c == 0), stop=False)
            stop_ps = accp.tile([1, B], F32, tag="acc")
            for c in range(8):
                nc.tensor.matmul(stop_ps[:], stopTx[c], xT[c][:],
                                 start=(c == 0), stop=False)
            for c in range(4):
                nc.tensor.matmul(stop_ps[:], stopTc[c], xcg[c],
                                 start=False, stop=False)

            # ---------- LSTM helper ----------
            def lstm(ihT_name, hhT_name, hh_rhs, extra_rhs, cT, lb, out_h, out_c, wpool, wtag):
                ih_all = load_chunked(wpool, ihT_name, 4 * UL, 8, tag=wtag + "_ih")
                hh_all = load_chunked(wpool, hhT_name, 4 * UL, 8, tag=wtag + "_hh")
                ih = [ih_all[:, 512 * c : 512 * (c + 1)] for c in range(8)]
                hh = [hh_all[:, 512 * c : 512 * (c + 1)] for c in range(8)]

                def gate_psum(g):
                    ps = gatp.tile([UL, B], F32, tag="gat")
                    n_mm = 16 + (8 if extra_rhs is not None else 0)
                    k = 0
                    for c in range(8):
                        nc.tensor.matmul(ps[:], ih_all[:, 512 * c + UL * g : 512 * c + UL * (g + 1)], xT[c][:],
                                         start=(k == 0), stop=(k == n_mm - 1)); k += 1
                    for c in range(8):
                        nc.tensor.matmul(ps[:], hh_all[:, 512 * c + UL * g : 512 * c + UL * (g + 1)], hh_rhs[c],
                                         start=(k == 0), stop=(k == n_mm - 1)); k += 1
                    if extra_rhs is not None:
                        for c in range(8):
                            nc.tensor.matmul(ps[:], ih_all[:, 512 * c + UL * g : 512 * c + UL * (g + 1)], extra_rhs[c],
                                             start=(k == 0), stop=(k == n_mm - 1)); k += 1
                    return ps

                # gate order: i, f, g, o
                def sig(g, col, tag):
                    ps = gate_psum(g)
                    s = wk.tile([UL, B], F32, tag=tag)
                    nc.scalar.activation(s[:], ps[:], AF.Tanh, bias=lb[:, col : col + 1], scale=0.5)
                    nc.vector.tensor_scalar(s[:], s[:], 0.5, 0.5, ALU.mult, ALU.add)
                    return s

                i_s = sig(0, 0, "i_s")
                f_s = sig(1, 1, "f_s")
                ps_g = gate_psum(2)
                g_s = wk.tile([UL, B], F32, tag="g_s")
                nc.scalar.activation(g_s[:], ps_g[:], AF.Tanh, bias=lb[:, 2:3])
                o_s = sig(3, 3, "o_s")
                tt1 = wk.tile([UL, B], F32, tag="tt1")
                nc.vector.tensor_tensor(tt1[:], f_s[:], cT[:], op=ALU.mult)
                tt2 = wk.tile([UL, B], F32, tag="tt2")
                nc.vector.tensor_tensor(tt2[:], i_s[:], g_s[:], op=ALU.mult)
                cn = wk.tile([UL, B], F32, tag="cn")
                nc.vector.tensor_tensor(cn[:], tt1[:], tt2[:], op=ALU.add)
                nc.sync.dma_start(out_c, cn[:])
                tc_ = wk.tile([UL, B], F32, tag="tc_")
                nc.scalar.activation(tc_[:], cn[:], AF.Tanh)
                hn = wk.tile([UL, B], F32R, tag=ihT_name + "hn")
                nc.vector.tensor_tensor(hn[:], o_s[:], tc_[:], op=ALU.mult)
                nc.sync.dma_start(out_h, hn[:])
                return hn

            h1T_all = load_chunked(hwp, "h1T", B, 8, tag="hT")
            h1T_sb = [h1T_all[:, B * c : B * (c + 1)] for c in range(8)]
            h1n = lstm("l1ihT", "l1hhT", h1T_sb, None, c1T, lb1, O["h1n"], O["c1n"], lwp, "lw1")

            # AllGather 2: h1n
            cc2i = dp.tile([UL, B], F32R, tag="cc2i")
            cc2o = dp.tile([LSTMD, B], F32R, tag="cc2o")
            nc.sync.dma_start(cc2i[:], h1n[:])
            if single:
                nc.sync.dma_start(cc2o[0:UL, :], cc2i[:, :])
            else:
                nc.gpsimd.collective_compute(
                    "AllGather", ALU.bypass, replica_groups=[list(range(NCORES))],
                    ins=[cc2i.opt()], outs=[cc2o.opt()],
                )
            h1f_all = wk.tile([128, 8 * B], F32R, tag="h1f", name="h1f_all")
            fullh = cc2o[:, :]
            srch = dataclasses.replace(
                fullh, offset=fullh.offset,
                ap=[[B, 128], [128 * B, 8], [1, B]],
            )
            nc.sync.dma_start(h1f_all[:], srch)
            h1f = [h1f_all[:, B * c : B * (c + 1)] for c in range(8)]
            for c in range(8):
                nc.tensor.matmul(mel_ps[:], melT[c], h1f[c], start=False, stop=False)
            for c in range(8):
                nc.tensor.matmul(stop_ps[:], stopTx[c], h1f[c], start=False, stop=False)

            h2T_all = load_chunked(hwp, "h2T", B, 8, tag="hT")
            h2T_sb = [h2T_all[:, B * c : B * (c + 1)] for c in range(8)]
            h2n = lstm("l2ihT", "l2hhT", h2T_sb, h1f, c2T, lb2, O["h2n"], O["c2n"], lw2p, "lw2")

            # AllGather 3: h2n
            cc3i = dp.tile([UL, B], F32R, tag="cc3i")
            cc3o = dp.tile([LSTMD, B], F32R, tag="cc3o")
            nc.sync.dma_start(cc3i[:], h2n[:])
            if single:
                nc.sync.dma_start(cc3o[0:UL, :], cc3i[:, :])
            else:
                nc.gpsimd.collective_compute(
                    "AllGather", ALU.bypass, replica_groups=[list(range(NCORES))],
                    ins=[cc3i.opt()], outs=[cc3o.opt()],
                )
            h2f_all = wk.tile([128, 8 * B], F32R, tag="h2f", name="h2f_all")
            fullh = cc3o[:, :]
            srch = dataclasses.replace(
                fullh, offset=fullh.offset,
                ap=[[B, 128], [128 * B, 8], [1, B]],
            )
            nc.sync.dma_start(h2f_all[:], srch)
            h2f = [h2f_all[:, B * c : B * (c + 1)] for c in range(8)]

            for c in range(8):
                nc.tensor.matmul(mel_ps[:], melT[c], h2f[c],
                                 start=False, stop=(c == 7))
            mel_sb = wk.tile([NMELS, B], F32, tag="mel_sb")
            nc.scalar.activation(mel_sb[:], mel_ps[:], AF.Copy)
            nc.sync.dma_start(O["melsT"], mel_sb[:])

            for c in range(8):
                nc.tensor.matmul(stop_ps[:], stopTx[c], h2f[c],
                                 start=False, stop=(c == 7))
            stop_sb = wk.tile([1, B], F32, tag="stop_sb")
            nc.scalar.activation(stop_sb[:], stop_ps[:], AF.Tanh, bias=stopb[0:1, 0:1], scale=0.5)
            nc.vector.tensor_scalar(stop_sb[:], stop_sb[:], 0.5, 0.5, ALU.mult, ALU.add)
            nc.sync.dma_start(O["stopT"], stop_sb[:])


def _build(single=False):
    nc = bacc.Bacc("TRN2", target_bir_lowering=False, debug=False,
                   num_devices=1 if single else NCORES)
    I, O = _declare(nc)
    with tile.TileContext(nc) as tc:
        _program(nc, tc, I, O, single=single)
    nc.compile()
    return nc


def _prep(inp):
    f = np.float32

    def T_(x):
        return np.ascontiguousarray(np.asarray(x).T.astype(f))

    enc = np.asarray(inp["encoder_seq"], dtype=f)
    proj = np.asarray(inp["encoder_seq_proj"], dtype=f)
    projT = np.ascontiguousarray(proj.transpose(0, 2, 1))
    cum = np.asarray(inp["cumulative"], dtype=f)
    cumP = np.zeros((B, T + KS - 1), dtype=f)
    cumP[:, (KS - 1) // 2 : (KS - 1) // 2 + T] = cum
    mask = (np.asarray(inp["chars"]) != 0).astype(f)

    LW = (inp["lsa_L"] @ inp["lsa_conv_w"][:, 0, :]).astype(f)       # [128, 31]
    Lb = (inp["lsa_L"] @ inp["lsa_conv_b"]).astype(f)                # [128]
    qbias = (inp["lsa_Wb"] + Lb).astype(f)
    v = np.asarray(inp["lsa_v"][0], dtype=f)
    VMSKa = np.zeros((DEC, BL * BL), dtype=f)
    for b in range(BL):
        VMSKa[:, BL * b + b] = v

    vecs = np.zeros((DEC, 17), dtype=f)
    vecs[:, 0] = inp["prenet_b1"][0:128]
    vecs[:, 1] = inp["prenet_b1"][128:256]
    vecs[:, 2] = inp["prenet_b2"][0:128]
    vecs[:, 3] = inp["prenet_b2"][128:256]
    vecs[:, 4] = 0.5 * (inp["gru_b_ih"][0:128] + inp["gru_b_hh"][0:128])
    vecs[:, 5] = 0.5 * (inp["gru_b_ih"][128:256] + inp["gru_b_hh"][128:256])
    vecs[:, 6] = inp["gru_b_hh"][256:384]
    vecs[:, 7] = inp["gru_b_ih"][256:384]
    vecs[:, 8] = qbias
    rb = np.asarray(inp["rnn_in_b"], dtype=f)
    for m in range(8):
        vecs[:, 9 + m] = rb[128 * m : 128 * (m + 1)]

    mel_used = inp["mel_w"][0::20, :].astype(f)                      # [80, 1024]
    stop_w = np.asarray(inp["stop_w"], dtype=f)

    common = {
        "w1T": T_(inp["prenet_w1"]),
        "w2T": T_(inp["prenet_w2"]),
        "gihT": T_(inp["gru_w_ih"]),
        "ghhT": T_(inp["gru_w_hh"]),
        "lsaWT": T_(inp["lsa_W"]),
        "LWT": T_(LW),
        "ident": np.eye(DEC, dtype=f),
        "VMSK": VMSKa,
        "rnnT": T_(inp["rnn_in_w"]),
        "melT": T_(mel_used),
        "stopTx": np.ascontiguousarray(stop_w[0, 0:LSTMD].astype(f)[:, None]),
        "stopTc": np.ascontiguousarray(stop_w[0, LSTMD:].astype(f)[:, None]),
        "h1T": T_(inp["rnn1_hidden"]),
        "h2T": T_(inp["rnn2_hidden"]),
        "vecs": vecs,
        "stopb": 0.5 * np.asarray(inp["stop_b"], dtype=f).reshape(1, 1),
    }

    c1T_full = T_(inp["rnn1_cell"])
    c2T_full = T_(inp["rnn2_cell"])
    lb_full1 = (np.asarray(inp["lstm1_b_ih"]) + np.asarray(inp["lstm1_b_hh"])).astype(f)
    lb_full2 = (np.asarray(inp["lstm2_b_ih"]) + np.asarray(inp["lstm2_b_hh"])).astype(f)
    w1ih = np.asarray(inp["lstm1_w_ih"], dtype=f)
    w1hh = np.asarray(inp["lstm1_w_hh"], dtype=f)
    w2ih = np.asarray(inp["lstm2_w_ih"], dtype=f)
    w2hh = np.asarray(inp["lstm2_w_hh"], dtype=f)

    in_maps = []
    for k in range(NCORES):
        bs = slice(BL * k, BL * (k + 1))
        us = np.concatenate([np.arange(g * LSTMD + UL * k, g * LSTMD + UL * (k + 1)) for g in range(4)])
        lb1 = np.stack(
            [0.5 * lb_full1[us[0:UL]], 0.5 * lb_full1[us[UL:2*UL]],
             lb_full1[us[2*UL:3*UL]], 0.5 * lb_full1[us[3*UL:4*UL]]], axis=1)
        lb2 = np.stack(
            [0.5 * lb_full2[us[0:UL]], 0.5 * lb_full2[us[UL:2*UL]],
             lb_full2[us[2*UL:3*UL]], 0.5 * lb_full2[us[3*UL:4*UL]]], axis=1)
        m = dict(common)
        m.update({
            "pinT": T_(inp["prenet_in"][bs]),
            "hT0": T_(inp["attn_hidden"][bs]),
            "ctxT0": T_(inp["context_vec"][bs]),
            "projT": np.ascontiguousarray(projT[bs]),
            "enc": np.ascontiguousarray(enc[bs]),
            "cumP": np.ascontiguousarray(cumP[bs]),
            "cum": np.ascontiguousarray(cum[bs]),
            "mask": np.ascontiguousarray(mask[bs]),
            "l1ihT": np.ascontiguousarray(w1ih[us, :].T),
            "l1hhT": np.ascontiguousarray(w1hh[us, :].T),
            "l2ihT": np.ascontiguousarray(w2ih[us, :].T),
            "l2hhT": np.ascontiguousarray(w2hh[us, :].T),
            "lb1": np.ascontiguousarray(lb1),
            "lb2": np.ascontiguousarray(lb2),
            "c1T": np.ascontiguousarray(c1T_full[UL * k : UL * (k + 1), :]),
            "c2T": np.ascontiguousarray(c2T_full[UL * k : UL * (k + 1), :]),
        })
        in_maps.append(m)
    return in_maps


def _assemble(results):
    f = np.float32
    mels = results[0]["melsT"].T.astype(f)[:, :, None]
    scores = np.concatenate([r["scores"] for r in results], axis=0)[:, None, :]
    attnh = np.concatenate([r["attnh"] for r in results], axis=0)
    ctx = np.concatenate([r["ctx"] for r in results], axis=0)
    h1 = np.concatenate([r["h1n"] for r in results], axis=0).T
    h2 = np.concatenate([r["h2n"] for r in results], axis=0).T
    c1 = np.concatenate([r["c1n"] for r in results], axis=0).T
    c2 = np.concatenate([r["c2n"] for r in results], axis=0).T
    stop = results[0]["stopT"].T.astype(f)
    cumn = np.concatenate([r["cumn"] for r in results], axis=0)
    return (
        np.ascontiguousarray(mels), np.ascontiguousarray(scores),
        np.ascontiguousarray(attnh), np.ascontiguousarray(h1),
        np.ascontiguousarray(h2), np.ascontiguousarray(c1),
        np.ascontiguousarray(c2), np.ascontiguousarray(ctx),
        np.ascontiguousarray(stop), np.ascontiguousarray(cumn),
    )


def kernel(**inputs):
    nc = _CACHE.get("nc")
    if nc is None:
        nc = _build()
        _CACHE["nc"] = nc
    in_maps = _prep(inputs)
    res = bass_utils.run_bass_kernel_spmd(nc, in_maps, core_ids=list(range(NCORES)))
    return _assemble(res.results)
